# revision 55
# baseline (speedup 1.0000x reference)
import sys
for _p in ('/opt/trn_rl_repo',):
    if _p not in sys.path:
        sys.path.insert(0, _p)

"""NLSGCRN cell Bass/Tile kernel for TRN2, batch-sharded SPMD over 8 cores.

Per-core shapes (b_loc = 4 batches):
  x [4,2000,32], state [4,2000,64], x_full [4,12,2000,48], emb [2000,16],
  pools gw/uw/gb/ub/gT/uT, out h [4,2000,64].

Strategy:
  A = exp(relu(emb emb^T)) (symmetric, bf16), d = rowsum, rinv = 1/d;
  s^k x = A-chunk matmuls with 1/d row-scale on PSUM eviction (natural
  [n, (b,c)] layout). Per-node grouped GEMM via the D-expansion:
  y[r,(d,o)] = Xg[r,ki] @ WP[ki,(d,o)], z[r,o] = bias + sum_d e[r,d]*y[r,(d,o)]
  with per-partition-scalar fused multiply-add (scalar_tensor_tensor) split
  DVE (direct from PSUM) / GPSIMD (after bf16 eviction by ACT).
  Window conv: xt = sum_t T[t] x_full[:,t] accumulated on GPSIMD, contracted
  with the same machinery (ki = 48).
"""

from contextlib import ExitStack

import concourse.bass as bass
import concourse.tile as tile
from concourse import mybir
from concourse._compat import with_exitstack

F32 = mybir.dt.float32
F32R = mybir.dt.float32r
BF16 = mybir.dt.bfloat16
AF = mybir.ActivationFunctionType
OP = mybir.AluOpType

B_LOC = 4
N = 2000
NCHUNK = 16           # ceil(2000/128)
NFULL = (NCHUNK - 1) * 128   # 1920
NPAD = NCHUNK * 128   # 2048
R = B_LOC * NPAD      # 8192 padded rows
NRC = R // 128        # 64 row-chunks
DIN, DOUT = 32, 64
CIN = 96
CW = 48
WLEN = 12
EMB = 16
K = 3

FLAT = N * CW // 128  # 750 (flat per-partition window elements)
DEBUG = False


def nlen(nch):
    return 128 if nch < NCHUNK - 1 else N - NFULL  # last = 80


def chunked_load(nc, dst, src, eng=None):
    """dst [128, NCHUNK, ...inner] <- src [2000, ...inner] splitting rows."""
    eng = eng or nc.sync
    inner = src.shape[1:]
    eng.dma_start(
        dst[:, 0 : NCHUNK - 1],
        src[0:NFULL].rearrange(
            "(c p) " + " ".join(f"i{j}" for j in range(len(inner)))
            + " -> p c " + " ".join(f"i{j}" for j in range(len(inner))),
            p=128,
        ),
    )
    eng.dma_start(dst[0 : N - NFULL, NCHUNK - 1], src[NFULL:N])


@with_exitstack
def build(ctx: ExitStack, tc: tile.TileContext, io: dict):
    nc = tc.nc

    io = {k: (v[:] if not isinstance(v, bass.AP) else v) for k, v in io.items()}
    x, state, x_full = io["x"], io["state"], io["x_full"]
    emb = io["node_embeddings"]
    out = io["out"]

    const = ctx.enter_context(tc.tile_pool(name="const", bufs=1))
    big = ctx.enter_context(tc.tile_pool(name="big", bufs=1))
    stage_scope = tc.tile_pool(name="stage", bufs=3)
    stage = stage_scope.__enter__()
    stage3_scope = tc.tile_pool(name="stage3", bufs=2)
    stage3 = stage3_scope.__enter__()

    # ================= constants / weights =================
    # Tb/eye/diag first: the t-contraction matmuls are PE's first work, so
    # nothing slow may precede the diag build on DVE.
    Tb = const.tile([128, 2, WLEN], F32)
    for w, name in ((0, "gT"), (1, "uT")):
        src = io[name][:]
        nc.sync.dma_start(
            Tb[:, w, :],
            bass.AP(tensor=src.tensor, offset=src.offset, ap=[[0, 128]] + list(src.ap)),
        )
    eye = const.tile([128, 128], BF16)
    nc.sync.dma_start(eye[:], io["eye128"][:])
    diag = const.tile([128, 2, WLEN, 128], BF16)
    for w in range(2):
        for t in range(WLEN):
            nc.vector.tensor_scalar(
                out=diag[:, w, t], in0=eye[:],
                scalar1=Tb[:, w, t : t + 1], scalar2=None, op0=OP.mult,
            )

    eexp = const.tile([128, NCHUNK, EMB], F32)
    nc.vector.memset(eexp[:], 0.0)
    chunked_load(nc, eexp, emb)

    # WPg [128, 3, 1024] bf16: rows 0:96 = c, cols (d,o) d-major.
    # Pad rows of the weight tiles can be anything on HW (the matching XgT
    # partitions are zero), but memset keeps the simulator's init-tracking
    # happy. (Memsets on ACT, which is idle until the A phase.)
    WPg = const.tile([128, K, EMB * 64], BF16)
    nc.scalar.memzero(WPg[:])
    WPu = const.tile([128, K, EMB * 32], BF16)
    nc.scalar.memzero(WPu[:])
    WWg = const.tile([128, EMB * 64], BF16)
    nc.scalar.memzero(WWg[:])
    WWu = const.tile([128, EMB * 32], BF16)
    nc.scalar.memzero(WWu[:])
    for k in range(K):
        wk = stage.tile([128, EMB, 64], F32, tag="stg")
        nc.sync.dma_start(wk[0:CIN], io["gw_pool"][:, k].rearrange("d c o -> c d o"))
        nc.vector.tensor_copy(
            WPg[0:CIN, k].rearrange("p (d o) -> p d o", d=EMB), wk[0:CIN]
        )
        wku = stage.tile([128, EMB, 32], F32, tag="stg")
        nc.sync.dma_start(wku[0:32], io["uw_pool"][:, k, 0:32, :].rearrange("d c o -> c d o"))
        nc.sync.dma_start(wku[64:128], io["uw_pool"][:, k, 32:96, :].rearrange("d c o -> c d o"))
        nc.vector.tensor_copy(
            WPu[0:32, k].rearrange("p (d o) -> p d o", d=EMB), wku[0:32]
        )
        nc.vector.tensor_copy(
            WPu[64:128, k].rearrange("p (d o) -> p d o", d=EMB), wku[64:128]
        )
    wg = stage.tile([128, EMB, 64], F32, tag="stg")
    nc.sync.dma_start(wg[0:CW], io["gw_win"].rearrange("d i o -> i d o"))
    nc.vector.tensor_copy(WWg[0:CW].rearrange("p (d o) -> p d o", d=EMB), wg[0:CW])
    wu = stage.tile([128, EMB, 32], F32, tag="stg")
    # rows 64:112 (matches packed XtT where xt_u.T sits at partitions 64:112)
    nc.sync.dma_start(wu[64 : 64 + CW], io["uw_win"].rearrange("d i o -> i d o"))
    nc.vector.tensor_copy(
        WWu[64 : 64 + CW].rearrange("p (d o) -> p d o", d=EMB), wu[64 : 64 + CW]
    )

    # Bias folding: ones-rows in the X panels / XtT meet these weight rows, so
    # PSUM accumulates emb@bias_pool without a separate bias pass.
    #   gate z bias -> WPg row 96 (X1 ones col 96, k=0)
    #   gate r bias -> WWg row 48 (XtT ones row 48)
    #   update graph bias -> WPu row 32 (CAND ones col 32, k=0)
    #   update window bias -> WWu row 48
    # gpsimd DMAs cast f32 -> bf16 in flight.
    def _row1(src2d):
        return bass.AP(tensor=src2d.tensor, offset=src2d.offset,
                       ap=[[0, 1]] + [list(d) for d in src2d.ap])

    nc.gpsimd.dma_start(
        WPg[96:97, 0, :].rearrange("p (d o) -> p d o", d=EMB),
        _row1(io["gb_pool"][:, 0:DOUT]),
    )
    nc.gpsimd.dma_start(
        WWg[48:49, :].rearrange("p (d o) -> p d o", d=EMB),
        _row1(io["gb_pool"][:, DOUT:]),
    )
    nc.gpsimd.dma_start(
        WPu[32:33, 0, :].rearrange("p (d o) -> p d o", d=EMB),
        _row1(io["ub_pool"][:, 0:32]),
    )
    nc.gpsimd.dma_start(
        WWu[48:49, :].rearrange("p (d o) -> p d o", d=EMB),
        _row1(io["ub_pool"][:, 32:64]),
    )

    # ================= window t-contraction on PE =================
    # xt[p, f] = sum_t T[t] * xf_t[p, f] as 12 accumulating matmuls with
    # stationary diag(T[t]) built from the host-provided identity. Runs first
    # so PE ramps up while embT/x/state DMAs land.
    FH = FLAT // 2  # 375, fits one PSUM bank in f32
    xt_scope = tc.tile_pool(name="xtp", bufs=1)
    xt_pool = xt_scope.__enter__()
    xt16 = xt_pool.tile([128, B_LOC, 2, FLAT], BF16, tag="xt16")
    with tc.tile_pool(name="psum_xt", bufs=2, space="PSUM") as pxt_pool, \
         tc.tile_pool(name="xfst", bufs=3) as xfst:
        for b in range(B_LOC):
            pts = {}
            for w in range(2):
                for half in range(2):
                    pts[w, half] = pxt_pool.tile(
                        [128, FH], F32, tag=f"xt{w}{half}", name=f"pxt{w}{half}"
                    )
            for tg in range(WLEN // 4):
                st = xfst.tile([128, 4, FLAT], BF16, tag="xf")
                nc.sync.dma_start(
                    st[:],
                    x_full[b, 4 * tg : 4 * tg + 4]
                    .rearrange("t n i -> t (n i)")
                    .rearrange("t (p f) -> p t f", p=128),
                )
                for tt in range(4):
                    t = 4 * tg + tt
                    for w in range(2):
                        for half in range(2):
                            nc.tensor.matmul(
                                pts[w, half][:], diag[:, w, t],
                                st[:, tt, half * FH : (half + 1) * FH],
                                start=(t == 0), stop=(t == WLEN - 1),
                            )
            for w in range(2):
                for half in range(2):
                    nc.scalar.copy(
                        xt16[:, b, w, half * FH : (half + 1) * FH], pts[w, half][:]
                    )

    # ================= A (pre phase PSUM) =================
    A = big.tile([128, NCHUNK, N], BF16, tag="A")
    rinv = const.tile([128, NCHUNK], F32)
    dsum_all = const.tile([128, NCHUNK], F32)

    with tc.tile_pool(name="prep", bufs=1) as prep:
        embT_raw = prep.tile([EMB, N], F32)
        nc.sync.dma_start(embT_raw[:], emb.rearrange("n d -> d n"))
        embT = prep.tile([EMB, N], F32R)
        nc.vector.tensor_copy(embT[:], embT_raw[:])
        with tc.tile_pool(name="psum_pre", bufs=2, space="PSUM") as psum_pre:
            for nch in range(NCHUNK):
                l = nlen(nch)
                nsl = slice(nch * 128, nch * 128 + l)
                pg = psum_pre.tile([128, N], F32, tag="pg")
                for mj in range(4):
                    m0 = mj * 512
                    mw = min(512, N - m0)
                    nc.tensor.matmul(
                        pg[:l, m0 : m0 + mw], embT[:, nsl],
                        embT[:, m0 : m0 + mw], start=True, stop=True,
                    )
                nc.scalar.activation(A[:l, nch, :], pg[:l, :], AF.Exp)
                nc.vector.tensor_scalar(
                    out=A[:l, nch, :], in0=A[:l, nch, :],
                    scalar1=1.0, scalar2=0.0, op0=OP.max, op1=OP.add,
                    accum_out=dsum_all[:l, nch : nch + 1],
                )
                nc.vector.reciprocal(rinv[:l, nch : nch + 1], dsum_all[:l, nch : nch + 1])

    # ================= x/state load; X1 [128, nch, b, 128] bf16 =================
    # SST keeps state resident for gating math (replaces per-chunk DMAs).
    # bf16: gpsimd-issued DMAs cast f32->bf16 in flight.
    SST = const.tile([128, NCHUNK, B_LOC, DOUT], BF16)
    X1 = big.tile([128, NCHUNK, B_LOC, 128], BF16, tag="slot1")
    nc.vector.memset(X1[:], 0.0)
    for b in range(B_LOC):
        xs = stage.tile([128, NCHUNK, DIN], F32, tag="stg")
        nc.vector.memset(xs[64:, NCHUNK - 1], 0.0)
        chunked_load(nc, xs, x[b])
        nc.vector.tensor_copy(X1[:, :, b, 0:DIN], xs[:])
        nc.vector.memset(SST[64:, NCHUNK - 1, b], 0.0)
        chunked_load(nc, SST[:, :, b], state[b], eng=nc.gpsimd)
        nc.scalar.copy(X1[:, :, b, DIN:CIN], SST[:, :, b])
    # ones col 96 meets WPg bias row 96 (k=0) -> gate z bias in PSUM
    nc.vector.memset(X1[:, :, :, 96:97], 1.0)

    # ================= window t-contraction (flat layout) =================
    # x_full[b, t] is accumulated in a flat [128, 750] view (2000*48 elems
    # row-major): elementwise sums don't care about layout, and flat DMAs are
    # fully contiguous. Results bounce through DRAM into packed [NPAD, 128]
    # transpose sources.
    dram = ctx.enter_context(tc.tile_pool(name="dram", bufs=6, space="DRAM"))
    HNCH = NCHUNK // 2   # half-panel: 8 n-chunks = 1024 rows

    def pair_panel_T(SRC, bp, h):
        """[128, 2, 1024] <- transposes of SRC[:, h*8:(h+1)*8, b, :] for the
        b-pair (2bp, 2bp+1), one DRAM bounce + one transpose for both."""
        t = xgt_pool.tile([128, 2 * HNCH * 128], BF16, tag="xgt")
        dp = dram.tile([2 * HNCH * 128, 128], BF16, tag="panh")
        dpv = dp.rearrange("(b c p) o -> p b c o", p=128, b=2)
        for j in range(2):
            nc.gpsimd.dma_start(
                dpv[:, j],
                SRC[:, h * HNCH : (h + 1) * HNCH, 2 * bp + j, :],
            )
        nc.sync.dma_start(t[:], dp[:], transpose=True)
        return t.rearrange("p (b f) -> p b f", b=2)

    # Materialized exactly: broadcast APs (stride-0 free dims) in DMAs leave
    # coverage holes on hardware -> undefined DRAM (NaN).
    zeros128 = const.tile([128, NCHUNK, 128], BF16)
    nc.vector.memset(zeros128[:], 0.0)
    dzero = dram.tile([NPAD, 128], BF16, tag="dzero")
    nc.sync.dma_start(dzero.rearrange("(c p) o -> p c o", p=128), zeros128[:])
    # ones column for XtT row 48 (bias folding: meets WWg/WWu bias rows).
    # Materialized exactly (no broadcast APs: a stride-0 mid free dim in a
    # DRAM->DRAM DMA lowers incorrectly).
    ones16 = const.tile([128, NCHUNK, 16], BF16)
    nc.vector.memset(ones16[:], 0.0)
    nc.vector.memset(ones16[:, :, 0:1], 1.0)
    dones = dram.tile([NPAD, 16], BF16, tag="dones")
    nc.sync.dma_start(dones.rearrange("(c p) o -> p c o", p=128), ones16[:])

    # pack via DRAM: XtT partitions 0:48 = xt_g.T, row 48 = ones, 64:112 = xt_u.T
    # dpan is a single persistent buffer: constant regions (ones col, zero
    # gaps, pad rows) are filled once; per-b only the data columns rewrite.
    XtT = big.tile([128, R], BF16, tag="XtT")
    dpan = dram.tile([NPAD, 128], BF16, tag="pan")
    nc.gpsimd.dma_start(dpan[0:N, CW:64], dones[0:N])
    nc.gpsimd.dma_start(dpan[0:N, 112:128], dzero[0:N, 0:16])
    nc.gpsimd.dma_start(dpan[N:NPAD, :], dzero[N:NPAD, :])
    for b in range(B_LOC):
        dflat = dram.tile([2, 128, FLAT], BF16, tag="dflat")
        nc.gpsimd.dma_start(dflat.rearrange("w p f -> p w f"), xt16[:, b])
        dfv = dflat.rearrange("w p f -> w (p f)").rearrange("w (n i) -> w n i", n=N)
        nc.gpsimd.dma_start(dpan[0:N, 0:CW], dfv[0])
        nc.gpsimd.dma_start(dpan[0:N, 64 : 64 + CW], dfv[1])
        nc.sync.dma_start(XtT[:, b * NPAD : (b + 1) * NPAD], dpan[:], transpose=True)
        if DEBUG and b == 0:
            nc.sync.dma_start(io["dbg_dpan"][:], dpan[:])
    if DEBUG:
        nc.sync.dma_start(io["dbg_xtt"][:], XtT[:])
    xt_scope.__exit__(None, None, None)
    stage3_scope.__exit__(None, None, None)
    stage_scope.__exit__(None, None, None)

    # ================= diffusion helper =================
    def diffuse(psum_pool, SRC, DST, c0, clen):
        for nch in range(NCHUNK):
            l = nlen(nch)
            ph = psum_pool.tile([128, B_LOC, clen], F32, tag="pdiff")
            for mi in range(NCHUNK):
                ml = nlen(mi)
                nc.tensor.matmul(
                    ph[:l], A[:ml, mi, nch * 128 : nch * 128 + l],
                    SRC[:ml, mi, :, c0 : c0 + clen],
                    start=(mi == 0), stop=(mi == NCHUNK - 1),
                )
            nc.scalar.activation(
                DST[:l, nch, :, c0 : c0 + clen], ph[:l],
                AF.Copy, scale=rinv[:l, nch : nch + 1],
            )

    # ================= gate diffusion =================
    X2 = big.tile([128, NCHUNK, B_LOC, 128], BF16, tag="slot2")
    X3 = big.tile([128, NCHUNK, B_LOC, 128], BF16, tag="slot3")
    nc.gpsimd.memset(X2[:], 0.0)
    nc.gpsimd.memset(X3[:], 0.0)
    with tc.tile_pool(name="psum_d1", bufs=3, space="PSUM") as psum_d1:
        diffuse(psum_d1, X1, X2, 0, CIN)
        diffuse(psum_d1, X2, X3, 0, CIN)

    # ================= shared y-phase machinery =================
    acc_pool = ctx.enter_context(tc.tile_pool(name="accp", bufs=3))
    ybf_pool = ctx.enter_context(tc.tile_pool(name="ybf", bufs=2))
    xgt_pool = ctx.enter_context(tc.tile_pool(name="xgt", bufs=5))

    # eviction engines rotate to spread PSUM->SBUF traffic (Pool cannot
    # read PSUM on hardware).
    def _ev_act(dst, src):
        nc.scalar.copy(dst, src)

    def _ev_dve(dst, src):
        nc.vector.tensor_copy(dst, src)

    EV_PAT = [_ev_act, _ev_act, _ev_dve]

    def dred_group(yv, owid, nch):
        """In-place d-reduction on yv [128, 4, EMB, owid] (4 = b-pair x blk):
        scale block d by e[p, d] (tensor_scalar, 4x mode), then a pairwise
        in-place add tree over d (tensor_tensor, 2x mode). Result lands in
        yv[:, :, 0, :]; bias is already folded into the matmul (ones rows)."""
        for d in range(EMB):
            nc.vector.tensor_scalar(
                out=yv[:, :, d], in0=yv[:, :, d],
                scalar1=eexp[:, nch, d : d + 1], scalar2=None, op0=OP.mult,
            )
        step = 1
        while step < EMB:
            # tail levels are small; Pool (idle during y phases) takes them
            eng = nc.vector if step < 4 else nc.gpsimd
            eng.tensor_tensor(
                out=yv[:, :, 0 : EMB : 2 * step],
                in0=yv[:, :, 0 : EMB : 2 * step],
                in1=yv[:, :, step : EMB : 2 * step],
                op=OP.add,
            )
            step *= 2

    # ================= gate y-GEMM + d-red + gating =================
    r_gate = big.tile([128, NCHUNK, B_LOC, DOUT], BF16, tag="r_gate")

    ev_i = 0
    dr_i = 0
    with tc.tile_pool(name="psum_yg", bufs=2, space="PSUM") as psum_yg, \
         tc.tile_pool(name="psum_yw", bufs=2, space="PSUM") as psum_yw:
        for h in range(2):
          for bp in range(2):
            pair = (2 * bp, 2 * bp + 1)
            xgb = [pair_panel_T(S, bp, h) for S in (X1, X2, X3)]
            for nch2 in range(HNCH):
                nch = h * HNCH + nch2
                l = nlen(nch)
                ybf2 = ybf_pool.tile([128, 2, 2, 1024], BF16, tag="ybf")
                for j, b in enumerate(pair):
                    r0 = b * NPAD + nch * 128
                    pg = psum_yg.tile([128, 1024], F32, tag="pyg")
                    for half in range(2):
                        for k in range(K):
                            nc.tensor.matmul(
                                pg[:, half * 512 : (half + 1) * 512],
                                xgb[k][:, j, nch2 * 128 : (nch2 + 1) * 128],
                                WPg[:, k, half * 512 : (half + 1) * 512],
                                start=(k == 0), stop=(k == K - 1),
                            )
                    pw = psum_yw.tile([128, 1024], F32, tag="pyw")
                    for half in range(2):
                        nc.tensor.matmul(
                            pw[:, half * 512 : (half + 1) * 512],
                            XtT[:, r0 : r0 + 128],
                            WWg[:, half * 512 : (half + 1) * 512],
                            start=True, stop=True,
                        )
                    EV_PAT[ev_i % len(EV_PAT)](ybf2[:, j, 0], pg[:])
                    EV_PAT[(ev_i + 1) % len(EV_PAT)](ybf2[:, j, 1], pw[:])
                    ev_i += 2
                    if DEBUG and h == 0 and bp == 0 and nch == 0 and j == 0:
                        nc.sync.dma_start(io["dbg"][:], ybf2[:, 0])
                yv = ybf2.rearrange("p b blk (d o) -> p (b blk) d o", d=EMB)
                dred_group(yv, DOUT, nch)
                for j, b in enumerate(pair):
                    ztile = acc_pool.tile([128, DOUT], BF16, tag="ztile")
                    nc.scalar.activation(ztile[:], ybf2[:, j, 0, 0:DOUT], AF.Sigmoid)
                    nc.scalar.activation(
                        r_gate[:, nch, b, :], ybf2[:, j, 1, 0:DOUT], AF.Sigmoid
                    )
                    # CAND panel reuses X1's slot: cols 0:32 keep x; stale cols
                    # 32:64 are neutralized by WPu's zero rows; z*state -> 64:128.
                    nc.gpsimd.tensor_mul(
                        X1[:, nch, b, 64:128], ztile[:], SST[:, nch, b]
                    )

    CAND = X1  # renamed: panels now hold [x | ones | stale | z*state]
    # ones col 32 meets WPu bias row 32 (k=0) -> update bias in PSUM (gate
    # y consumed the old state col 32 already; stale cols 33:64 are
    # neutralized by WPu's zero rows).
    nc.vector.memset(X1[:, :, :, 32:33], 1.0)

    # ================= update diffusion =================
    # C2/C3 reuse X2/X3 slots: cols 0:32 already hold diffused-x hops; zero
    # 32:64; diffusion writes 64:128.
    C2, C3 = X2, X3
    with tc.tile_pool(name="psum_d2", bufs=3, space="PSUM") as psum_d2:
        diffuse(psum_d2, CAND, C2, 64, DOUT)
        diffuse(psum_d2, C2, C3, 64, DOUT)

    # ================= update y-GEMM + d-red + output =================
    with tc.tile_pool(name="psum_yu", bufs=3, space="PSUM") as psum_yu, \
         tc.tile_pool(name="psum_uw", bufs=3, space="PSUM") as psum_uw:
        for h in range(2):
          for bp in range(2):
            pair = (2 * bp, 2 * bp + 1)
            xgb = [pair_panel_T(S, bp, h) for S in (CAND, C2, C3)]
            for nch2 in range(HNCH):
                nch = h * HNCH + nch2
                l = nlen(nch)
                ubf_full = ybf_pool.tile([128, 2, 2, 1024], BF16, tag="ybf")
                ubf2 = ubf_full[:, :, :, 0:512]
                for j, b in enumerate(pair):
                    r0 = b * NPAD + nch * 128
                    pu = psum_yu.tile([128, 512], F32, tag="pyu")
                    for k in range(K):
                        nc.tensor.matmul(
                            pu[:], xgb[k][:, j, nch2 * 128 : (nch2 + 1) * 128],
                            start=(k == 0), stop=(k == K - 1), rhs=WPu[:, k, :],
                        )
                    uw = psum_uw.tile([128, 512], F32, tag="puw")
                    nc.tensor.matmul(
                        uw[:], XtT[:, r0 : r0 + 128], WWu[:],
                        start=True, stop=True,
                    )
                    EV_PAT[ev_i % len(EV_PAT)](ubf2[:, j, 0], pu[:])
                    EV_PAT[(ev_i + 1) % len(EV_PAT)](ubf2[:, j, 1], uw[:])
                    ev_i += 2
                uv = ubf2.rearrange("p b blk (d o) -> p (b blk) d o", d=EMB)
                dred_group(uv, 32, nch)
                tmp2 = acc_pool.tile([128, 2, DOUT], F32, tag="tmp2")
                for j, b in enumerate(pair):
                    hc = acc_pool.tile([128, DOUT], F32, tag="hc")
                    nc.scalar.activation(
                        hc.rearrange("p (blk o) -> p blk o", blk=2),
                        ubf2[:, j, :, 0:32], AF.Tanh,
                    )
                    nc.gpsimd.tensor_sub(tmp2[:, j], SST[:, nch, b], hc[:])
                    nc.gpsimd.tensor_mul(tmp2[:, j], tmp2[:, j], r_gate[:, nch, b, :])
                    nc.gpsimd.tensor_add(tmp2[:, j], tmp2[:, j], hc[:])
                nc.gpsimd.dma_start(
                    out[pair[0] : pair[0] + 2, nch * 128 : nch * 128 + l, :]
                    .rearrange("b n o -> n b o"),
                    tmp2[:l],
                )


def make_io(nc):
    io = {}
    io["x"] = nc.dram_tensor("x", [B_LOC, N, DIN], F32, kind="ExternalInput")
    io["state"] = nc.dram_tensor("state", [B_LOC, N, DOUT], F32, kind="ExternalInput")
    io["x_full"] = nc.dram_tensor("x_full", [B_LOC, WLEN, N, CW], BF16, kind="ExternalInput")
    io["eye128"] = nc.dram_tensor("eye128", [128, 128], BF16, kind="ExternalInput")
    io["node_embeddings"] = nc.dram_tensor("node_embeddings", [N, EMB], F32, kind="ExternalInput")
    io["gw_pool"] = nc.dram_tensor("gw_pool", [EMB, K, CIN, 64], F32, kind="ExternalInput")
    io["gw_win"] = nc.dram_tensor("gw_win", [EMB, CW, 64], F32, kind="ExternalInput")
    io["gb_pool"] = nc.dram_tensor("gb_pool", [EMB, 2 * DOUT], F32, kind="ExternalInput")
    io["gT"] = nc.dram_tensor("gT", [WLEN], F32, kind="ExternalInput")
    io["uw_pool"] = nc.dram_tensor("uw_pool", [EMB, K, CIN, 32], F32, kind="ExternalInput")
    io["uw_win"] = nc.dram_tensor("uw_win", [EMB, CW, 32], F32, kind="ExternalInput")
    io["ub_pool"] = nc.dram_tensor("ub_pool", [EMB, DOUT], F32, kind="ExternalInput")
    io["uT"] = nc.dram_tensor("uT", [WLEN], F32, kind="ExternalInput")
    io["out"] = nc.dram_tensor("out", [B_LOC, N, DOUT], F32, kind="ExternalOutput")
    if DEBUG:
        io["dbg"] = nc.dram_tensor("dbg", [128, 2, EMB * 64], BF16, kind="ExternalOutput")
        io["dbg_dpan"] = nc.dram_tensor("dbg_dpan", [NPAD, 128], BF16, kind="ExternalOutput")
        io["dbg_xtt"] = nc.dram_tensor("dbg_xtt", [128, R], BF16, kind="ExternalOutput")
    return io


def build_module(debug=False):
    from concourse import bacc

    nc = bacc.Bacc("TRN2", target_bir_lowering=False, debug=debug)
    io = make_io(nc)
    with tile.TileContext(nc) as tc:
        build(tc, io)
    nc.finalize()
    return nc


# ======================= harness wrapper =======================
import numpy as _np

N_CORES = 8
_CACHE = {}


def _get_module():
    if "nc" not in _CACHE:
        _CACHE["nc"] = build_module()
    return _CACHE["nc"]


def kernel(**inputs):
    """Full-input entry point: shards over batch across 8 NeuronCores."""
    import ml_dtypes

    nc = _get_module()
    from concourse.bass_utils import run_bass_kernel_spmd

    bf16 = ml_dtypes.bfloat16
    xb = _np.ascontiguousarray(inputs["x"], dtype=_np.float32)
    sb = _np.ascontiguousarray(inputs["state"], dtype=_np.float32)
    xf = _np.ascontiguousarray(
        _np.asarray(inputs["x_full"], dtype=_np.float32).astype(bf16)
    )
    rep = {
        k: _np.ascontiguousarray(inputs[k], dtype=_np.float32)
        for k in ("node_embeddings", "gw_pool", "gw_win", "gb_pool", "gT",
                  "uw_pool", "uw_win", "ub_pool", "uT")
    }
    rep["eye128"] = _np.eye(128, dtype=bf16)
    in_maps = []
    for i in range(N_CORES):
        m = dict(rep)
        m["x"] = xb[i * B_LOC : (i + 1) * B_LOC]
        m["state"] = sb[i * B_LOC : (i + 1) * B_LOC]
        m["x_full"] = xf[i * B_LOC : (i + 1) * B_LOC]
        in_maps.append(m)
    res = run_bass_kernel_spmd(nc, in_maps, core_ids=list(range(N_CORES)))
    return _np.concatenate([res.results[i]["out"] for i in range(N_CORES)], axis=0)



# revision 56
# speedup vs baseline: 1.0706x; 1.0706x over previous
import sys
for _p in ('/opt/trn_rl_repo',):
    if _p not in sys.path:
        sys.path.insert(0, _p)

"""NLSGCRN cell Bass/Tile kernel for TRN2, batch-sharded SPMD over 8 cores.

Per-core shapes (b_loc = 4 batches):
  x [4,2000,32], state [4,2000,64], x_full [4,12,2000,48], emb [2000,16],
  pools gw/uw/gb/ub/gT/uT, out h [4,2000,64].

Strategy:
  A = exp(relu(emb emb^T)) (symmetric, bf16), d = rowsum, rinv = 1/d;
  s^k x = A-chunk matmuls with 1/d row-scale on PSUM eviction (natural
  [n, (b,c)] layout). Per-node grouped GEMM via the D-expansion:
  y[r,(d,o)] = Xg[r,ki] @ WP[ki,(d,o)], z[r,o] = bias + sum_d e[r,d]*y[r,(d,o)]
  with per-partition-scalar fused multiply-add (scalar_tensor_tensor) split
  DVE (direct from PSUM) / GPSIMD (after bf16 eviction by ACT).
  Window conv: xt = sum_t T[t] x_full[:,t] accumulated on GPSIMD, contracted
  with the same machinery (ki = 48).
"""

from contextlib import ExitStack

import concourse.bass as bass
import concourse.tile as tile
from concourse import mybir
from concourse._compat import with_exitstack

F32 = mybir.dt.float32
F32R = mybir.dt.float32r
BF16 = mybir.dt.bfloat16
AF = mybir.ActivationFunctionType
OP = mybir.AluOpType

B_LOC = 4
N = 2000
NCHUNK = 16           # ceil(2000/128)
NFULL = (NCHUNK - 1) * 128   # 1920
NPAD = NCHUNK * 128   # 2048
R = B_LOC * NPAD      # 8192 padded rows
NRC = R // 128        # 64 row-chunks
DIN, DOUT = 32, 64
CIN = 96
CW = 48
WLEN = 12
EMB = 16
K = 3

FLAT = N * CW // 128  # 750 (flat per-partition window elements)
DEBUG = False


def nlen(nch):
    return 128 if nch < NCHUNK - 1 else N - NFULL  # last = 80


def chunked_load(nc, dst, src, eng=None):
    """dst [128, NCHUNK, ...inner] <- src [2000, ...inner] splitting rows."""
    eng = eng or nc.sync
    inner = src.shape[1:]
    eng.dma_start(
        dst[:, 0 : NCHUNK - 1],
        src[0:NFULL].rearrange(
            "(c p) " + " ".join(f"i{j}" for j in range(len(inner)))
            + " -> p c " + " ".join(f"i{j}" for j in range(len(inner))),
            p=128,
        ),
    )
    eng.dma_start(dst[0 : N - NFULL, NCHUNK - 1], src[NFULL:N])


@with_exitstack
def build(ctx: ExitStack, tc: tile.TileContext, io: dict):
    nc = tc.nc

    io = {k: (v[:] if not isinstance(v, bass.AP) else v) for k, v in io.items()}
    x, state, x_full = io["x"], io["state"], io["x_full"]
    emb = io["node_embeddings"]
    out = io["out"]

    const = ctx.enter_context(tc.tile_pool(name="const", bufs=1))
    big = ctx.enter_context(tc.tile_pool(name="big", bufs=1))
    stage_scope = tc.tile_pool(name="stage", bufs=3)
    stage = stage_scope.__enter__()
    stage3_scope = tc.tile_pool(name="stage3", bufs=2)
    stage3 = stage3_scope.__enter__()

    # ================= constants / weights =================
    # Tb/eye/diag first: the t-contraction matmuls are PE's first work, so
    # nothing slow may precede the diag build on DVE.
    Tb = const.tile([128, 2, WLEN], F32)
    for w, name in ((0, "gT"), (1, "uT")):
        src = io[name][:]
        nc.sync.dma_start(
            Tb[:, w, :],
            bass.AP(tensor=src.tensor, offset=src.offset, ap=[[0, 128]] + list(src.ap)),
        )
    eye = const.tile([128, 128], BF16)
    nc.sync.dma_start(eye[:], io["eye128"][:])
    diag = const.tile([128, 2, WLEN, 128], BF16)
    for w in range(2):
        for t in range(WLEN):
            nc.vector.tensor_scalar(
                out=diag[:, w, t], in0=eye[:],
                scalar1=Tb[:, w, t : t + 1], scalar2=None, op0=OP.mult,
            )

    eexp = const.tile([128, NCHUNK, EMB], F32)
    nc.vector.memset(eexp[:], 0.0)
    chunked_load(nc, eexp, emb)

    # WPg [128, 3, 1024] bf16: rows 0:96 = c, cols (d,o) d-major.
    # Pad rows of the weight tiles can be anything on HW (the matching XgT
    # partitions are zero), but memset keeps the simulator's init-tracking
    # happy. (Memsets on ACT, which is idle until the A phase.)
    WPg = const.tile([128, K, EMB * 64], BF16)
    nc.scalar.memzero(WPg[:])
    WPu = const.tile([128, K, EMB * 32], BF16)
    nc.scalar.memzero(WPu[:])
    WWg = const.tile([128, EMB * 64], BF16)
    nc.scalar.memzero(WWg[:])
    WWu = const.tile([128, EMB * 32], BF16)
    nc.scalar.memzero(WWu[:])
    for k in range(K):
        wk = stage.tile([128, EMB, 64], F32, tag="stg")
        nc.sync.dma_start(wk[0:CIN], io["gw_pool"][:, k].rearrange("d c o -> c d o"))
        nc.vector.tensor_copy(
            WPg[0:CIN, k].rearrange("p (d o) -> p d o", d=EMB), wk[0:CIN]
        )
        wku = stage.tile([128, EMB, 32], F32, tag="stg")
        nc.sync.dma_start(wku[0:32], io["uw_pool"][:, k, 0:32, :].rearrange("d c o -> c d o"))
        nc.sync.dma_start(wku[64:128], io["uw_pool"][:, k, 32:96, :].rearrange("d c o -> c d o"))
        nc.vector.tensor_copy(
            WPu[0:32, k].rearrange("p (d o) -> p d o", d=EMB), wku[0:32]
        )
        nc.vector.tensor_copy(
            WPu[64:128, k].rearrange("p (d o) -> p d o", d=EMB), wku[64:128]
        )
    wg = stage.tile([128, EMB, 64], F32, tag="stg")
    nc.sync.dma_start(wg[0:CW], io["gw_win"].rearrange("d i o -> i d o"))
    nc.vector.tensor_copy(WWg[0:CW].rearrange("p (d o) -> p d o", d=EMB), wg[0:CW])
    wu = stage.tile([128, EMB, 32], F32, tag="stg")
    # rows 64:112 (matches packed XtT where xt_u.T sits at partitions 64:112)
    nc.sync.dma_start(wu[64 : 64 + CW], io["uw_win"].rearrange("d i o -> i d o"))
    nc.vector.tensor_copy(
        WWu[64 : 64 + CW].rearrange("p (d o) -> p d o", d=EMB), wu[64 : 64 + CW]
    )

    # Bias folding: ones-rows in the X panels / XtT meet these weight rows, so
    # PSUM accumulates emb@bias_pool without a separate bias pass.
    #   gate z bias -> WPg row 96 (X1 ones col 96, k=0)
    #   gate r bias -> WWg row 48 (XtT ones row 48)
    #   update graph bias -> WPu row 32 (CAND ones col 32, k=0)
    #   update window bias -> WWu row 48
    # gpsimd DMAs cast f32 -> bf16 in flight.
    def _row1(src2d):
        return bass.AP(tensor=src2d.tensor, offset=src2d.offset,
                       ap=[[0, 1]] + [list(d) for d in src2d.ap])

    nc.gpsimd.dma_start(
        WPg[96:97, 0, :].rearrange("p (d o) -> p d o", d=EMB),
        _row1(io["gb_pool"][:, 0:DOUT]),
    )
    nc.gpsimd.dma_start(
        WWg[48:49, :].rearrange("p (d o) -> p d o", d=EMB),
        _row1(io["gb_pool"][:, DOUT:]),
    )
    nc.gpsimd.dma_start(
        WPu[32:33, 0, :].rearrange("p (d o) -> p d o", d=EMB),
        _row1(io["ub_pool"][:, 0:32]),
    )
    nc.gpsimd.dma_start(
        WWu[48:49, :].rearrange("p (d o) -> p d o", d=EMB),
        _row1(io["ub_pool"][:, 32:64]),
    )

    # ================= window t-contraction on PE =================
    # xt[p, f] = sum_t T[t] * xf_t[p, f] as 12 accumulating matmuls with
    # stationary diag(T[t]) built from the host-provided identity. Runs first
    # so PE ramps up while embT/x/state DMAs land.
    FH = FLAT // 2  # 375, fits one PSUM bank in f32
    xt_scope = tc.tile_pool(name="xtp", bufs=1)
    xt_pool = xt_scope.__enter__()
    xt16 = xt_pool.tile([128, B_LOC, 2, FLAT], BF16, tag="xt16")
    with tc.tile_pool(name="psum_xt", bufs=2, space="PSUM") as pxt_pool, \
         tc.tile_pool(name="xfst", bufs=3) as xfst:
        for b in range(B_LOC):
            pts = {}
            for w in range(2):
                for half in range(2):
                    pts[w, half] = pxt_pool.tile(
                        [128, FH], F32, tag=f"xt{w}{half}", name=f"pxt{w}{half}"
                    )
            for tg in range(WLEN // 4):
                st = xfst.tile([128, 4, FLAT], BF16, tag="xf")
                nc.sync.dma_start(
                    st[:],
                    x_full[b, 4 * tg : 4 * tg + 4]
                    .rearrange("t n i -> t (n i)")
                    .rearrange("t (p f) -> p t f", p=128),
                )
                for tt in range(4):
                    t = 4 * tg + tt
                    for w in range(2):
                        for half in range(2):
                            nc.tensor.matmul(
                                pts[w, half][:], diag[:, w, t],
                                st[:, tt, half * FH : (half + 1) * FH],
                                start=(t == 0), stop=(t == WLEN - 1),
                            )
            for w in range(2):
                for half in range(2):
                    nc.scalar.copy(
                        xt16[:, b, w, half * FH : (half + 1) * FH], pts[w, half][:]
                    )

    # ================= A (pre phase PSUM) =================
    A = big.tile([128, NCHUNK, N], BF16, tag="A")
    rinv = const.tile([128, NCHUNK], F32)
    dsum_all = const.tile([128, NCHUNK], F32)

    with tc.tile_pool(name="prep", bufs=1) as prep:
        embT_raw = prep.tile([EMB, N], F32)
        nc.sync.dma_start(embT_raw[:], emb.rearrange("n d -> d n"))
        embT = prep.tile([EMB, N], F32R)
        nc.vector.tensor_copy(embT[:], embT_raw[:])
        with tc.tile_pool(name="psum_pre", bufs=2, space="PSUM") as psum_pre:
            for nch in range(NCHUNK):
                l = nlen(nch)
                nsl = slice(nch * 128, nch * 128 + l)
                pg = psum_pre.tile([128, N], F32, tag="pg")
                for mj in range(4):
                    m0 = mj * 512
                    mw = min(512, N - m0)
                    nc.tensor.matmul(
                        pg[:l, m0 : m0 + mw], embT[:, nsl],
                        embT[:, m0 : m0 + mw], start=True, stop=True,
                    )
                nc.scalar.activation(A[:l, nch, :], pg[:l, :], AF.Exp)
                nc.vector.tensor_scalar(
                    out=A[:l, nch, :], in0=A[:l, nch, :],
                    scalar1=1.0, scalar2=0.0, op0=OP.max, op1=OP.add,
                    accum_out=dsum_all[:l, nch : nch + 1],
                )
                nc.vector.reciprocal(rinv[:l, nch : nch + 1], dsum_all[:l, nch : nch + 1])

    # ================= x/state load; X1 [128, nch, b, 128] bf16 =================
    # SST keeps state resident for gating math (replaces per-chunk DMAs).
    # bf16: gpsimd-issued DMAs cast f32->bf16 in flight.
    SST = const.tile([128, NCHUNK, B_LOC, DOUT], BF16)
    X1 = big.tile([128, NCHUNK, B_LOC, 128], BF16, tag="slot1")
    nc.vector.memset(X1[:], 0.0)
    for b in range(B_LOC):
        xs = stage.tile([128, NCHUNK, DIN], F32, tag="stg")
        nc.vector.memset(xs[64:, NCHUNK - 1], 0.0)
        chunked_load(nc, xs, x[b])
        nc.vector.tensor_copy(X1[:, :, b, 0:DIN], xs[:])
        nc.vector.memset(SST[64:, NCHUNK - 1, b], 0.0)
        chunked_load(nc, SST[:, :, b], state[b], eng=nc.gpsimd)
        nc.scalar.copy(X1[:, :, b, DIN:CIN], SST[:, :, b])
    # ones col 96 meets WPg bias row 96 (k=0) -> gate z bias in PSUM
    nc.vector.memset(X1[:, :, :, 96:97], 1.0)

    # ================= window t-contraction (flat layout) =================
    # x_full[b, t] is accumulated in a flat [128, 750] view (2000*48 elems
    # row-major): elementwise sums don't care about layout, and flat DMAs are
    # fully contiguous. Results bounce through DRAM into packed [NPAD, 128]
    # transpose sources.
    dram = ctx.enter_context(tc.tile_pool(name="dram", bufs=6, space="DRAM"))
    HNCH = NCHUNK // 2   # half-panel: 8 n-chunks = 1024 rows

    def pair_panel_T(SRC, bp, h):
        """[128, 2, 1024] <- transposes of SRC[:, h*8:(h+1)*8, b, :] for the
        b-pair (2bp, 2bp+1), one DRAM bounce + one transpose for both."""
        t = xgt_pool.tile([128, 2 * HNCH * 128], BF16, tag="xgt")
        dp = dram.tile([2 * HNCH * 128, 128], BF16, tag="panh")
        dpv = dp.rearrange("(b c p) o -> p b c o", p=128, b=2)
        for j in range(2):
            nc.gpsimd.dma_start(
                dpv[:, j],
                SRC[:, h * HNCH : (h + 1) * HNCH, 2 * bp + j, :],
            )
        nc.sync.dma_start(t[:], dp[:], transpose=True)
        return t.rearrange("p (b f) -> p b f", b=2)

    # Materialized exactly: broadcast APs (stride-0 free dims) in DMAs leave
    # coverage holes on hardware -> undefined DRAM (NaN).
    zeros128 = const.tile([128, NCHUNK, 128], BF16)
    nc.vector.memset(zeros128[:], 0.0)
    dzero = dram.tile([NPAD, 128], BF16, tag="dzero")
    nc.sync.dma_start(dzero.rearrange("(c p) o -> p c o", p=128), zeros128[:])
    # ones column for XtT row 48 (bias folding: meets WWg/WWu bias rows).
    # Materialized exactly (no broadcast APs: a stride-0 mid free dim in a
    # DRAM->DRAM DMA lowers incorrectly).
    ones16 = const.tile([128, NCHUNK, 16], BF16)
    nc.vector.memset(ones16[:], 0.0)
    nc.vector.memset(ones16[:, :, 0:1], 1.0)
    dones = dram.tile([NPAD, 16], BF16, tag="dones")
    nc.sync.dma_start(dones.rearrange("(c p) o -> p c o", p=128), ones16[:])

    # pack via DRAM: XtT partitions 0:48 = xt_g.T, row 48 = ones, 64:112 = xt_u.T
    # dpan is a single persistent buffer: constant regions (ones col, zero
    # gaps, pad rows) are filled once; per-b only the data columns rewrite.
    XtT = big.tile([128, R], BF16, tag="XtT")
    dpan = dram.tile([NPAD, 128], BF16, tag="pan")
    nc.gpsimd.dma_start(dpan[0:N, CW:64], dones[0:N])
    nc.gpsimd.dma_start(dpan[0:N, 112:128], dzero[0:N, 0:16])
    nc.gpsimd.dma_start(dpan[N:NPAD, :], dzero[N:NPAD, :])
    for b in range(B_LOC):
        dflat = dram.tile([2, 128, FLAT], BF16, tag="dflat")
        nc.gpsimd.dma_start(dflat.rearrange("w p f -> p w f"), xt16[:, b])
        dfv = dflat.rearrange("w p f -> w (p f)").rearrange("w (n i) -> w n i", n=N)
        nc.gpsimd.dma_start(dpan[0:N, 0:CW], dfv[0])
        nc.gpsimd.dma_start(dpan[0:N, 64 : 64 + CW], dfv[1])
        nc.sync.dma_start(XtT[:, b * NPAD : (b + 1) * NPAD], dpan[:], transpose=True)
        if DEBUG and b == 0:
            nc.sync.dma_start(io["dbg_dpan"][:], dpan[:])
    if DEBUG:
        nc.sync.dma_start(io["dbg_xtt"][:], XtT[:])
    xt_scope.__exit__(None, None, None)
    stage3_scope.__exit__(None, None, None)
    stage_scope.__exit__(None, None, None)

    # ================= diffusion helper =================
    def diffuse(psum_pool, SRC, DST, c0, clen):
        for nch in range(NCHUNK):
            l = nlen(nch)
            ph = psum_pool.tile([128, B_LOC, clen], F32, tag="pdiff")
            for mi in range(NCHUNK):
                ml = nlen(mi)
                nc.tensor.matmul(
                    ph[:l], A[:ml, mi, nch * 128 : nch * 128 + l],
                    SRC[:ml, mi, :, c0 : c0 + clen],
                    start=(mi == 0), stop=(mi == NCHUNK - 1),
                )
            nc.scalar.activation(
                DST[:l, nch, :, c0 : c0 + clen], ph[:l],
                AF.Copy, scale=rinv[:l, nch : nch + 1],
            )

    # ================= gate diffusion =================
    X2 = big.tile([128, NCHUNK, B_LOC, 128], BF16, tag="slot2")
    X3 = big.tile([128, NCHUNK, B_LOC, 128], BF16, tag="slot3")
    nc.gpsimd.memset(X2[:], 0.0)
    nc.gpsimd.memset(X3[:], 0.0)
    with tc.tile_pool(name="psum_d1", bufs=3, space="PSUM") as psum_d1:
        diffuse(psum_d1, X1, X2, 0, CIN)
        diffuse(psum_d1, X2, X3, 0, CIN)

    # ================= shared y-phase machinery =================
    acc_pool = ctx.enter_context(tc.tile_pool(name="accp", bufs=3))
    ybf_pool = ctx.enter_context(tc.tile_pool(name="ybf", bufs=2))
    xgt_pool = ctx.enter_context(tc.tile_pool(name="xgt", bufs=5))

    # eviction engines rotate to spread PSUM->SBUF traffic (Pool cannot
    # read PSUM on hardware).
    def _ev_act(dst, src):
        nc.scalar.copy(dst, src)

    def _ev_dve(dst, src):
        nc.vector.tensor_copy(dst, src)

    EV_PAT = [_ev_act, _ev_act, _ev_dve]

    def dred_group(yv, owid, nch):
        """In-place d-reduction on yv [128, 4, EMB, owid] (4 = b-pair x blk):
        scale block d by e[p, d] (tensor_scalar, 4x mode), then a pairwise
        in-place add tree over d (tensor_tensor, 2x mode). Result lands in
        yv[:, :, 0, :]; bias is already folded into the matmul (ones rows)."""
        for d in range(EMB):
            nc.vector.tensor_scalar(
                out=yv[:, :, d], in0=yv[:, :, d],
                scalar1=eexp[:, nch, d : d + 1], scalar2=None, op0=OP.mult,
            )
        step = 1
        while step < EMB:
            eng = nc.vector
            eng.tensor_tensor(
                out=yv[:, :, 0 : EMB : 2 * step],
                in0=yv[:, :, 0 : EMB : 2 * step],
                in1=yv[:, :, step : EMB : 2 * step],
                op=OP.add,
            )
            step *= 2

    # ================= gate y-GEMM + d-red + gating =================
    r_gate = big.tile([128, NCHUNK, B_LOC, DOUT], BF16, tag="r_gate")

    ev_i = 0
    dr_i = 0
    with tc.tile_pool(name="psum_yg", bufs=2, space="PSUM") as psum_yg, \
         tc.tile_pool(name="psum_yw", bufs=2, space="PSUM") as psum_yw:
        for h in range(2):
          for bp in range(2):
            pair = (2 * bp, 2 * bp + 1)
            xgb = [pair_panel_T(S, bp, h) for S in (X1, X2, X3)]
            for nch2 in range(HNCH):
                nch = h * HNCH + nch2
                l = nlen(nch)
                ybf2 = ybf_pool.tile([128, 2, 2, 1024], BF16, tag="ybf")
                for j, b in enumerate(pair):
                    r0 = b * NPAD + nch * 128
                    pg = psum_yg.tile([128, 1024], F32, tag="pyg")
                    for half in range(2):
                        for k in range(K):
                            nc.tensor.matmul(
                                pg[:, half * 512 : (half + 1) * 512],
                                xgb[k][:, j, nch2 * 128 : (nch2 + 1) * 128],
                                WPg[:, k, half * 512 : (half + 1) * 512],
                                start=(k == 0), stop=(k == K - 1),
                            )
                    pw = psum_yw.tile([128, 1024], F32, tag="pyw")
                    for half in range(2):
                        nc.tensor.matmul(
                            pw[:, half * 512 : (half + 1) * 512],
                            XtT[:, r0 : r0 + 128],
                            WWg[:, half * 512 : (half + 1) * 512],
                            start=True, stop=True,
                        )
                    EV_PAT[ev_i % len(EV_PAT)](ybf2[:, j, 0], pg[:])
                    EV_PAT[(ev_i + 1) % len(EV_PAT)](ybf2[:, j, 1], pw[:])
                    ev_i += 2
                    if DEBUG and h == 0 and bp == 0 and nch == 0 and j == 0:
                        nc.sync.dma_start(io["dbg"][:], ybf2[:, 0])
                yv = ybf2.rearrange("p b blk (d o) -> p (b blk) d o", d=EMB)
                dred_group(yv, DOUT, nch)
                for j, b in enumerate(pair):
                    ztile = acc_pool.tile([128, DOUT], BF16, tag="ztile")
                    nc.scalar.activation(ztile[:], ybf2[:, j, 0, 0:DOUT], AF.Sigmoid)
                    nc.scalar.activation(
                        r_gate[:, nch, b, :], ybf2[:, j, 1, 0:DOUT], AF.Sigmoid
                    )
                    # CAND panel reuses X1's slot: cols 0:32 keep x; stale cols
                    # 32:64 are neutralized by WPu's zero rows; z*state -> 64:128.
                    nc.gpsimd.tensor_mul(
                        X1[:, nch, b, 64:128], ztile[:], SST[:, nch, b]
                    )

    CAND = X1  # renamed: panels now hold [x | ones | stale | z*state]
    # ones col 32 meets WPu bias row 32 (k=0) -> update bias in PSUM (gate
    # y consumed the old state col 32 already; stale cols 33:64 are
    # neutralized by WPu's zero rows).
    nc.vector.memset(X1[:, :, :, 32:33], 1.0)

    # ================= update diffusion =================
    # C2/C3 reuse X2/X3 slots: cols 0:32 already hold diffused-x hops; zero
    # 32:64; diffusion writes 64:128.
    C2, C3 = X2, X3
    with tc.tile_pool(name="psum_d2", bufs=3, space="PSUM") as psum_d2:
        diffuse(psum_d2, CAND, C2, 64, DOUT)
        diffuse(psum_d2, C2, C3, 64, DOUT)

    # ================= update y-GEMM + d-red + output =================
    with tc.tile_pool(name="psum_yu", bufs=3, space="PSUM") as psum_yu, \
         tc.tile_pool(name="psum_uw", bufs=3, space="PSUM") as psum_uw:
        for h in range(2):
          for bp in range(2):
            pair = (2 * bp, 2 * bp + 1)
            xgb = [pair_panel_T(S, bp, h) for S in (CAND, C2, C3)]
            for nch2 in range(HNCH):
                nch = h * HNCH + nch2
                l = nlen(nch)
                ubf_full = ybf_pool.tile([128, 2, 2, 1024], BF16, tag="ybf")
                ubf2 = ubf_full[:, :, :, 0:512]
                for j, b in enumerate(pair):
                    r0 = b * NPAD + nch * 128
                    pu = psum_yu.tile([128, 512], F32, tag="pyu")
                    for k in range(K):
                        nc.tensor.matmul(
                            pu[:], xgb[k][:, j, nch2 * 128 : (nch2 + 1) * 128],
                            start=(k == 0), stop=(k == K - 1), rhs=WPu[:, k, :],
                        )
                    uw = psum_uw.tile([128, 512], F32, tag="puw")
                    nc.tensor.matmul(
                        uw[:], XtT[:, r0 : r0 + 128], WWu[:],
                        start=True, stop=True,
                    )
                    EV_PAT[ev_i % len(EV_PAT)](ubf2[:, j, 0], pu[:])
                    EV_PAT[(ev_i + 1) % len(EV_PAT)](ubf2[:, j, 1], uw[:])
                    ev_i += 2
                uv = ubf2.rearrange("p b blk (d o) -> p (b blk) d o", d=EMB)
                dred_group(uv, 32, nch)
                tmp2 = acc_pool.tile([128, 2, DOUT], F32, tag="tmp2")
                for j, b in enumerate(pair):
                    hc = acc_pool.tile([128, DOUT], F32, tag="hc")
                    nc.scalar.activation(
                        hc.rearrange("p (blk o) -> p blk o", blk=2),
                        ubf2[:, j, :, 0:32], AF.Tanh,
                    )
                    nc.gpsimd.tensor_sub(tmp2[:, j], SST[:, nch, b], hc[:])
                    nc.gpsimd.tensor_mul(tmp2[:, j], tmp2[:, j], r_gate[:, nch, b, :])
                    nc.gpsimd.tensor_add(tmp2[:, j], tmp2[:, j], hc[:])
                nc.gpsimd.dma_start(
                    out[pair[0] : pair[0] + 2, nch * 128 : nch * 128 + l, :]
                    .rearrange("b n o -> n b o"),
                    tmp2[:l],
                )


def make_io(nc):
    io = {}
    io["x"] = nc.dram_tensor("x", [B_LOC, N, DIN], F32, kind="ExternalInput")
    io["state"] = nc.dram_tensor("state", [B_LOC, N, DOUT], F32, kind="ExternalInput")
    io["x_full"] = nc.dram_tensor("x_full", [B_LOC, WLEN, N, CW], BF16, kind="ExternalInput")
    io["eye128"] = nc.dram_tensor("eye128", [128, 128], BF16, kind="ExternalInput")
    io["node_embeddings"] = nc.dram_tensor("node_embeddings", [N, EMB], F32, kind="ExternalInput")
    io["gw_pool"] = nc.dram_tensor("gw_pool", [EMB, K, CIN, 64], F32, kind="ExternalInput")
    io["gw_win"] = nc.dram_tensor("gw_win", [EMB, CW, 64], F32, kind="ExternalInput")
    io["gb_pool"] = nc.dram_tensor("gb_pool", [EMB, 2 * DOUT], F32, kind="ExternalInput")
    io["gT"] = nc.dram_tensor("gT", [WLEN], F32, kind="ExternalInput")
    io["uw_pool"] = nc.dram_tensor("uw_pool", [EMB, K, CIN, 32], F32, kind="ExternalInput")
    io["uw_win"] = nc.dram_tensor("uw_win", [EMB, CW, 32], F32, kind="ExternalInput")
    io["ub_pool"] = nc.dram_tensor("ub_pool", [EMB, DOUT], F32, kind="ExternalInput")
    io["uT"] = nc.dram_tensor("uT", [WLEN], F32, kind="ExternalInput")
    io["out"] = nc.dram_tensor("out", [B_LOC, N, DOUT], F32, kind="ExternalOutput")
    if DEBUG:
        io["dbg"] = nc.dram_tensor("dbg", [128, 2, EMB * 64], BF16, kind="ExternalOutput")
        io["dbg_dpan"] = nc.dram_tensor("dbg_dpan", [NPAD, 128], BF16, kind="ExternalOutput")
        io["dbg_xtt"] = nc.dram_tensor("dbg_xtt", [128, R], BF16, kind="ExternalOutput")
    return io


def build_module(debug=False):
    from concourse import bacc

    nc = bacc.Bacc("TRN2", target_bir_lowering=False, debug=debug)
    io = make_io(nc)
    with tile.TileContext(nc) as tc:
        build(tc, io)
    nc.finalize()
    return nc


# ======================= harness wrapper =======================
import numpy as _np

N_CORES = 8
_CACHE = {}


def _get_module():
    if "nc" not in _CACHE:
        _CACHE["nc"] = build_module()
    return _CACHE["nc"]


def kernel(**inputs):
    """Full-input entry point: shards over batch across 8 NeuronCores."""
    import ml_dtypes

    nc = _get_module()
    from concourse.bass_utils import run_bass_kernel_spmd

    bf16 = ml_dtypes.bfloat16
    xb = _np.ascontiguousarray(inputs["x"], dtype=_np.float32)
    sb = _np.ascontiguousarray(inputs["state"], dtype=_np.float32)
    xf = _np.ascontiguousarray(
        _np.asarray(inputs["x_full"], dtype=_np.float32).astype(bf16)
    )
    rep = {
        k: _np.ascontiguousarray(inputs[k], dtype=_np.float32)
        for k in ("node_embeddings", "gw_pool", "gw_win", "gb_pool", "gT",
                  "uw_pool", "uw_win", "ub_pool", "uT")
    }
    rep["eye128"] = _np.eye(128, dtype=bf16)
    in_maps = []
    for i in range(N_CORES):
        m = dict(rep)
        m["x"] = xb[i * B_LOC : (i + 1) * B_LOC]
        m["state"] = sb[i * B_LOC : (i + 1) * B_LOC]
        m["x_full"] = xf[i * B_LOC : (i + 1) * B_LOC]
        in_maps.append(m)
    res = run_bass_kernel_spmd(nc, in_maps, core_ids=list(range(N_CORES)))
    return _np.concatenate([res.results[i]["out"] for i in range(N_CORES)], axis=0)

